# revision 43
# baseline (speedup 1.0000x reference)
"""Bass/Trainium2 kernel for nn_Attention_66297115181568 (sparse_attention).

Strategy: head-parallel across 8 NeuronCores; core h computes head h
end-to-end and its 64-row slice of the Wo projection. The host sums the
8 partial (512, 512) outputs (the tensor-parallel all-reduce) and adds bo.

Final version (v8 = 75.1us vs the 76.1us v1 baseline re-measured in
this session; stated harness baseline 91.4us). Key points:
  1. rel-term matmuls in fp8 DoubleRow perf mode (measured 216ns per
     [128, 2, 512] matmul at cruise = 2 fp8/partition/cycle, 2x the
     bf16 col-tiled scheme). DoubleRow is ISA-incompatible with PE
     column tiling, so each (block, g-pair) stationary is a full
     128-column [128, 2, 128] window, zero outside its block strip.
  2. IB=8 block geometry (partitions = 8 i-rows x 16 d): only NG=4
     d-groups -> 4 repl matmuls + 4 strided strip-muls build all the
     stationaries. Windows for the 16 blocks of a tile are overlapped
     in a [1928]-elem plane per (gp, t, u): window bl starts at 120*bl,
     strip bl at 128*bl, so each window holds exactly its own strip
     (at column 8*bl+c) and shared zeros elsewhere.
  3. qpad zero-fill as f32-bitcast memsets (4B/cycle instead of 1),
     split gpsimd/DVE, emitted first so it is off every critical path.
  4. DMA: inputs then all rel pieces (2MB, last one split 2x1MB) on the
     sync queue as one FIFO (sustains ~423 GB/s); only outputs (bf16)
     on the scalar queue, so main-loop ACT work never delays a rel
     doorbell. All rel tiles stay resident (16MB of SBUF).
  5. Softmax: exp(dots/64) with no max-subtraction (logits are O(1));
     the x64 logit scale rides in the k-projection activation (KF);
     exp in 4 column chunks, all 4 w-transposes before the 4 attnV
     matmuls so the PE never stalls on a DVE copy; transposes in bf16.
  6. Projections stay bf16 (an fp8-DoubleRow variant measured SLOWER:
     exposed LDWEIGHTS on half-width stationaries, +8us tensor busy).
"""

import sys

sys.path.insert(0, "/opt/trn_rl_repo")

from contextlib import ExitStack

import numpy as np
import ml_dtypes

import concourse.bass as bass
import concourse.tile as tile
from concourse import mybir
from concourse.ap import AP

# problem dims (hardcoded per spec)
B, N, DIM, H, D = 1, 512, 512, 8, 64
INNER = H * D
N_CORES = 8
P = 128                 # SBUF partitions
NT = N // P             # 4 row tiles
KT = DIM // P           # 4 contraction tiles for projections
IB = 8                  # i-block rows per partition-group
NBT = P // IB           # 16 blocks per row tile
DG = P // IB            # 16 d's per d-group
NG = D // DG            # 4 d-groups
NGP = NG // 2           # 2 d-group pairs (DoubleRow)
WS = P - IB             # 120: window stride in the packed plane
PW = WS * (NBT - 1) + P  # 1928: packed plane width
SCALE = D ** -0.5
NEG_BIG = 1.0e36
RELSCALE = 64.0         # host scales rel by this before fp8 cast
KF = SCALE * RELSCALE   # folded into the k/krot projection activation

f32 = mybir.dt.float32
bf16 = mybir.dt.bfloat16
fp8 = mybir.dt.float8e4
AX = mybir.AxisListType
ALU = mybir.AluOpType
AF = mybir.ActivationFunctionType
PM = mybir.MatmulPerfMode


def legalize_multi_waits(nc):
    """This walrus build supports only one sync-wait per instruction; hoist
    extra waits onto same-engine NoOps placed immediately before."""
    nid = 0
    for fn in nc.m.functions:
        for bb in fn.blocks:
            new = []
            changed = False
            for inst in bb.instructions:
                si = inst.sync_info
                waits = si.on_wait if si is not None else []
                if len(waits) > 1:
                    for w in waits[:-1]:
                        nop = mybir.InstNoOp(name=f"I-waitfix-{nid}")
                        nid += 1
                        nop.engine = inst.engine
                        nop.sync_info = mybir.SyncInfo(on_wait=[w], on_update=[])
                        new.append(nop)
                    si.on_wait = [waits[-1]]
                    inst.sync_info = si
                    changed = True
                new.append(inst)
            if changed:
                bb.instructions = new


def build_nc(use_mask=True, use_bias=False):
    nc = bass.Bass()

    xt_ext = nc.declare_dram_parameter("xt", [P, KT * N], bf16, isOutput=False)
    # combined projection weights [model-dim, (q|k|v)] swizzled
    wqkv_ext = nc.declare_dram_parameter("wqkv", [P, KT * 3 * D], bf16,
                                         isOutput=False)
    biasrow_ext = nc.declare_dram_parameter("biasrow", [1, 3 * D], f32,
                                            isOutput=False)
    maskrow_ext = nc.declare_dram_parameter("maskrow", [1, N], f32,
                                            isOutput=False)
    wo_ext = nc.declare_dram_parameter("wo", [D, DIM], bf16, isOutput=False)
    # RoPE constants in [n, d] layout: [p, nt, {cos, sin_rot}, {q, k}, d];
    # the k half carries KF, sin_rot carries the rotate-half signs
    cssn_ext = nc.declare_dram_parameter("cssn", [P, NT * 2 * 2 * D], bf16,
                                         isOutput=False)
    tconst_ext = nc.declare_dram_parameter("tconst", [D, NG * P], fp8,
                                           isOutput=False)
    m512_ext = nc.declare_dram_parameter("m512", [P, N], bf16, isOutput=False)
    identf_ext = nc.declare_dram_parameter("identf", [P, P], bf16, isOutput=False)
    # rel stream: [tile, p=(i_l*16+d_l), (block16, g4, j)] fp8; 1MB pieces
    rel_ext = nc.declare_dram_parameter("rel", [NT, P, NBT * NG * N], fp8,
                                        isOutput=False)
    out_ext = nc.declare_dram_parameter("out", [N, DIM], bf16, isOutput=True)

    with tile.TileContext(nc) as tc, ExitStack() as ctx:
        dma = nc.sync      # HWDGE queue 1: the whole rel stream
        dma2 = nc.scalar   # HWDGE queue 2: inputs first, then outputs
        consts = ctx.enter_context(tc.tile_pool(name="consts", bufs=1))
        pro = ctx.enter_context(tc.tile_pool(name="pro", bufs=1))
        relp = ctx.enter_context(tc.tile_pool(name="relp", bufs=1))
        smp = ctx.enter_context(tc.tile_pool(name="smp", bufs=2))
        smallp = ctx.enter_context(tc.tile_pool(name="smallp", bufs=2))
        outp = ctx.enter_context(tc.tile_pool(name="outp", bufs=2))
        op = ctx.enter_context(tc.tile_pool(name="op", bufs=1))
        psA = ctx.enter_context(
            tc.tile_pool(name="psA", bufs=2, space=bass.MemorySpace.PSUM))
        psB = ctx.enter_context(
            tc.tile_pool(name="psB", bufs=2, space=bass.MemorySpace.PSUM))
        psW = ctx.enter_context(
            tc.tile_pool(name="psW", bufs=2, space=bass.MemorySpace.PSUM))
        psV = ctx.enter_context(
            tc.tile_pool(name="psV", bufs=1, space=bass.MemorySpace.PSUM))
        psO = ctx.enter_context(
            tc.tile_pool(name="psO", bufs=1, space=bass.MemorySpace.PSUM))

        # ---- qpad zero-fill FIRST: f32-bitcast memsets split across
        # gpsimd and DVE (both otherwise idle at t0) ----
        qpad = consts.tile([P, NGP, 2, NT, PW], fp8, name="qpad")
        half0 = qpad[:, 0:1].rearrange("p a t u w -> p (a t u w)").bitcast(f32)
        half1 = qpad[:, 1:2].rearrange("p a t u w -> p (a t u w)").bitcast(f32)
        nc.gpsimd.memset(half0, 0.0)
        nc.vector.memset(half1, 0.0)

        # ---- inputs on the sync queue, ahead of the rel stream (v1-proven
        # arrangement: one queue, inputs as the FIFO prefix; the scalar
        # queue carries only outputs so main-loop ACT work never delays a
        # transfer doorbell) ----
        xt_sb = pro.tile([P, KT, N], bf16)
        dma.dma_start(out=xt_sb[:], in_=xt_ext.rearrange("p (u n) -> p u n", u=KT))
        wqkv_sb = pro.tile([P, KT, 3 * D], bf16)
        dma.dma_start(out=wqkv_sb[:],
                      in_=wqkv_ext.rearrange("p (u m) -> p u m", u=KT))
        biasrow_sb = consts.tile([1, 3 * D], f32)
        if use_bias:
            dma.dma_start(out=biasrow_sb[:], in_=biasrow_ext[:])
        maskrow_sb = consts.tile([1, N], f32)
        if use_mask:
            dma.dma_start(out=maskrow_sb[:], in_=maskrow_ext[:])
        cssn_sb = consts.tile([P, NT, 2, 2, D], bf16)
        dma.dma_start(out=cssn_sb[:],
                      in_=cssn_ext.rearrange("p (u s c d) -> p u s c d",
                                             u=NT, s=2, c=2))
        tconst_sb = consts.tile([D, NG, P], fp8)
        dma.dma_start(out=tconst_sb[:],
                      in_=tconst_ext.rearrange("d (g p) -> d g p", g=NG))
        m512_sb = consts.tile([P, N], bf16)
        dma.dma_start(out=m512_sb[:], in_=m512_ext[:])
        identb = consts.tile([P, P], bf16)
        dma.dma_start(out=identb[:], in_=identf_ext[:])
        wo_sb = consts.tile([D, DIM], bf16)
        dma.dma_start(out=wo_sb[:], in_=wo_ext[:])
        ones_sb = consts.tile([1, N], f32)
        nc.vector.memset(ones_sb, 1.0)

        # ---- rel stream: 2MB pieces (2 per row tile), all on the sync
        # queue; the final piece split into two 1MB halves for a short
        # PE tail ----
        rel_tiles = {}
        hpt = NBT * NG * N // 2    # free elems per half-tile piece (8 blocks)
        for it in range(NT):
            for pc in range(2):
                if (it, pc) == (NT - 1, 1):
                    rpa = relp.tile([P, hpt // 2], fp8, name="rel7a")
                    dma.dma_start(out=rpa[:],
                                  in_=rel_ext[it, :, pc * hpt:pc * hpt + hpt // 2])
                    rpb = relp.tile([P, hpt // 2], fp8, name="rel7b")
                    dma.dma_start(out=rpb[:],
                                  in_=rel_ext[it, :, pc * hpt + hpt // 2:(pc + 1) * hpt])
                    rel_tiles[(it, pc)] = (rpa, rpb)
                else:
                    rp = relp.tile([P, hpt], fp8, name=f"rel{it}_{pc}")
                    dma.dma_start(out=rp[:],
                                  in_=rel_ext[it, :, pc * hpt:(pc + 1) * hpt])
                    rel_tiles[(it, pc)] = rp

        def rel_mv(it, bl, gp):
            """moving AP [128, 2, 512] for (tile, block16, g-pair)"""
            pc, bi = divmod(bl, NBT // 2)   # half-tile piece, block-in-piece
            rp = rel_tiles[(it, pc)]
            if isinstance(rp, tuple):
                # halves split blocks 8-11 / 12-15 of tile 3
                rp = rp[0] if bi < 4 else rp[1]
                bi = bi % 4
            off = (bi * NG + 2 * gp) * N
            full = rp[:]
            return AP(full.tensor, full.offset + off,
                      [list(full.ap[0]), [N, 2], [1, N]])

        # ---- projections in [n, d] layout: stationary = xT chunk
        # [128, 128], moving = Wqkv [128, 192]. One PSUM bank per n-tile
        # holds q|k|v rows; v copies straight into its attnV layout, q|k
        # copy to SBUF for the DVE RoPE. No qrot/krot projections: the
        # rotate-half happens via strided DVE views. ----
        qk_nd = pro.tile([P, NT, 2, D], bf16)
        v_sb = pro.tile([P, NT, D], bf16)
        for nt in range(NT):
            ps_q = psA.tile([P, N], f32, tag="big")
            for u in range(KT):
                nc.tensor.matmul(ps_q[:, 0:3 * D],
                                 xt_sb[:, u, nt * P:(nt + 1) * P],
                                 wqkv_sb[:, u, :],
                                 start=(u == 0), stop=(u == KT - 1))
            nc.scalar.copy(qk_nd[:, nt, :, :], ps_q[:, 0:2 * D])
            nc.scalar.copy(v_sb[:, nt, :], ps_q[:, 2 * D:3 * D])
        if use_bias:
            nc.vector.tensor_add(
                qk_nd[:],
                qk_nd[:],
                biasrow_sb[0:1, 0:2 * D].rearrange("o (c d) -> o c d", c=2)
                .partition_broadcast(P).unsqueeze(1).broadcast_to(
                    (P, NT, 2, D)))
            nc.vector.tensor_add(
                v_sb[:], v_sb[:],
                biasrow_sb[0:1, 2 * D:3 * D].partition_broadcast(P)
                .unsqueeze(1).broadcast_to((P, NT, D)))

        # ---- RoPE on DVE in [n, d] layout: q' = q*cos + rot(q)*sin with
        # rot via d-strided views (sin_rot carries the signs, k rows the
        # KF logit scale) ----
        qkp_nd = pro.tile([P, NT, 2, D], bf16)
        t2 = pro.tile([P, NT, 2, D], bf16, tag="ropet2")
        nc.vector.tensor_mul(qkp_nd[:], qk_nd[:], cssn_sb[:, :, 0, :, :])
        nc.vector.tensor_mul(t2[:, :, :, 0::2], qk_nd[:, :, :, 1::2],
                             cssn_sb[:, :, 1, :, 0::2])
        nc.vector.tensor_mul(t2[:, :, :, 1::2], qk_nd[:, :, :, 0::2],
                             cssn_sb[:, :, 1, :, 1::2])
        nc.vector.tensor_add(qkp_nd[:], qkp_nd[:], t2[:])

        # ---- transpose q'/k' back to [d, n] for the repl/qk matmuls ----
        qk_T = pro.tile([D, 2, N], bf16)
        for nt in range(NT):
            for c in range(2):
                pq = psW.tile([P, P], bf16, tag="tp")
                nc.tensor.transpose(pq[0:D, :], qkp_nd[:, nt, c, :], identb[:])
                nc.scalar.copy(qk_T[:, c, nt * P:(nt + 1) * P], pq[0:D, :])
        qpT = qk_T[:, 0, :]
        kpT = qk_T[:, 1, :]

        # ---- Qpad strips: Rep_g[p, n] = q'T[g*16 + p%16, n]; strips go to
        # plane (gp=g//2, t=g%2): [u, 128*bl + c] (c = p//16 in-strip col),
        # read back as windows [u, 120*bl : 120*bl+128]. ----
        qpad_full = qpad[:]
        ppair = list(qpad_full.ap[0])

        def emit_strip(g):
            ps_rep = psB.tile([P, N], f32, tag="rep")
            nc.tensor.matmul(ps_rep[:], tconst_sb[:, g, :], qpT,
                             start=True, stop=True)
            strip_out = AP(qpad_full.tensor,
                           qpad_full.offset + g * (NT * PW),
                           [ppair, [PW, NT], [P, NBT], [1, IB]])
            nc.vector.tensor_mul(
                strip_out,
                ps_rep.rearrange("p (u v c) -> p u v c", u=NT, v=NBT)[:],
                m512_sb.rearrange("p (u v c) -> p u v c", u=NT, v=NBT)[:])

        # strips first (4 repl + 4 strided muls pipeline PE/DVE; the DVE
        # finishes each pair before the PE's first matmuls need it)
        for g in range(NG):
            emit_strip(g)

        # ---- main loop ----
        o_tiles = [op.tile([P, DIM], bf16, name=f"o{it}") for it in range(NT)]
        for it in range(NT):
            dots_ps = psA.tile([P, N], f32, tag="big")
            nc.tensor.matmul(dots_ps[:], qpT[:, it * P:(it + 1) * P], kpT,
                             start=True, stop=False, skip_group_check=True)
            if use_mask:
                nc.tensor.matmul(dots_ps[:], ones_sb[:, 0:P], maskrow_sb[:],
                                 start=False, stop=False, skip_group_check=True)
            for bl in range(NBT):
                for gp in range(NGP):
                    nc.tensor.matmul(
                        dots_ps[:],
                        qpad[:, gp, :, it, WS * bl:WS * bl + P],
                        rel_mv(it, bl, gp),
                        start=False,
                        stop=(bl == NBT - 1 and gp == NGP - 1),
                        perf_mode=PM.DoubleRow,
                        skip_group_check=True)

            # softmax: unnormalized exp(dots/RELSCALE) in bf16, no
            # max-subtraction; 4 column chunks. All 4 transposes run before
            # the attnV matmuls so the PE never stalls on a DVE copy.
            w_sm = smp.tile([P, N], bf16, tag="w_sm")
            rowsum4 = smallp.tile([P, NT], f32, tag="rowsum4")
            wT_sb = outp.tile([P, NT, P], bf16, tag="wT_sb")
            attn_ps = psV.tile([D, P], f32, tag="attn")
            for jt in range(NT):
                nc.scalar.activation(w_sm[:, jt * P:(jt + 1) * P],
                                     dots_ps[:, jt * P:(jt + 1) * P], AF.Exp,
                                     scale=1.0 / RELSCALE,
                                     accum_out=rowsum4[:, jt:jt + 1])
                wp = psW.tile([P, P], bf16, tag="tp")
                nc.tensor.transpose(wp[:], w_sm[:, jt * P:(jt + 1) * P],
                                    identb[:])
                nc.vector.tensor_copy(wT_sb[:, jt, :], wp[:])
            for jt in range(NT):
                nc.tensor.matmul(attn_ps[:], v_sb[:, jt, :], wT_sb[:, jt, :],
                                 start=(jt == 0), stop=(jt == NT - 1))
            rowsum = smallp.tile([P, 1], f32, tag="rowsum")
            nc.vector.tensor_reduce(rowsum[:], rowsum4[:], AX.X, ALU.add)
            rcp = smallp.tile([P, 1], f32, tag="rcp")
            nc.vector.reciprocal(rcp[:], rowsum[:])

            attn_sb = outp.tile([D, P], bf16, tag="attn_sb")
            nc.scalar.copy(attn_sb[:], attn_ps[:])
            out_ps = psO.tile([P, DIM], f32, tag="out")
            nc.tensor.matmul(out_ps[:], attn_sb[:], wo_sb[:], start=True, stop=True)
            nc.scalar.activation(o_tiles[it][:], out_ps[:], AF.Copy, scale=rcp[:])
            dma2.dma_start(out=out_ext[it * P:(it + 1) * P, :], in_=o_tiles[it][:])

    legalize_multi_waits(nc)
    return nc


_NC_CACHE = None
TRACE = False
LAST_RESULT = None


def _get_nc(use_mask, use_bias):
    global _NC_CACHE
    if _NC_CACHE is None or _NC_CACHE[1] != (use_mask, use_bias):
        _NC_CACHE = (build_nc(use_mask, use_bias), (use_mask, use_bias))
    return _NC_CACHE[0]


def kernel(**inputs):
    x = np.asarray(inputs["x"], dtype=np.float32)
    mask = np.asarray(inputs["mask"])
    rope = np.asarray(inputs["rope"], dtype=np.float32)
    rel_pos = np.asarray(inputs["rel_pos"], dtype=np.float32)
    Wq = np.asarray(inputs["Wq"], dtype=np.float32)
    bq = np.asarray(inputs["bq"], dtype=np.float32)
    Wk = np.asarray(inputs["Wk"], dtype=np.float32)
    bk = np.asarray(inputs["bk"], dtype=np.float32)
    Wv = np.asarray(inputs["Wv"], dtype=np.float32)
    bv = np.asarray(inputs["bv"], dtype=np.float32)
    Wo = np.asarray(inputs["Wo"], dtype=np.float32)
    bo = np.asarray(inputs["bo"], dtype=np.float32)

    use_mask = not bool(np.asarray(mask).all())
    use_bias = bool(np.any(bq) or np.any(bk) or np.any(bv))
    nc = _get_nc(use_mask, use_bias)

    def swz(a):  # [K, M] -> [p, (u, M)] with K = (u, p)
        k, m = a.shape
        return np.ascontiguousarray(
            a.reshape(KT, P, m).transpose(1, 0, 2).reshape(P, KT * m))

    xT = swz(x.reshape(N, DIM).T.astype(np.float32)).astype(ml_dtypes.bfloat16)
    maskrow = ((mask.reshape(1, N).astype(np.float32)) - 1.0) * NEG_BIG

    # cssn[p, nt, {cos, sin_rot}, {q, k}, d]: cos/sin(rope[nt*128+p, d]);
    # the k half carries KF; sin_rot flips sign on even d (rotate-half)
    cos_nd = np.cos(rope).reshape(NT, P, D).transpose(1, 0, 2)  # [P, NT, D]
    sin_nd = np.sin(rope).reshape(NT, P, D).transpose(1, 0, 2)
    sgn = np.where(np.arange(D) % 2 == 0, -1.0, 1.0)[None, None, :]
    cosf = cos_nd[:, :, None, :] * np.array([1.0, KF])[None, None, :, None]
    sinr = (sin_nd * sgn)[:, :, None, :] * np.array([1.0, KF])[None, None, :, None]
    cssn = np.stack([cosf, sinr], axis=2)          # [P, NT, 2, 2, D]
    cssn = cssn.reshape(P, NT * 2 * 2 * D).astype(ml_dtypes.bfloat16)

    # T[d, g, p] = (d == g*DG + p%DG); m512[p, n] = (n%IB == p//DG)
    d_i = np.arange(D)[:, None, None]
    g_i = np.arange(NG)[None, :, None]
    p_i = np.arange(P)[None, None, :]
    tconst = (d_i == g_i * DG + p_i % DG).astype(np.float32)
    tconst = tconst.reshape(D, NG * P).astype(ml_dtypes.float8_e4m3)
    p_2 = np.arange(P)[:, None]
    n_2 = np.arange(N)[None, :]
    m512 = ((n_2 % IB) == (p_2 // DG)).astype(np.float32)
    m512 = m512.astype(ml_dtypes.bfloat16)

    identf = np.eye(P, dtype=np.float32).astype(ml_dtypes.bfloat16)

    # rel: [h, it, p=(i_l*DG+d_l), (block16, g, j)] fp8, scaled by RELSCALE
    rel8 = (rel_pos[0] * RELSCALE).astype(ml_dtypes.float8_e4m3)
    # [h, (it, b16, i_l), j, (g, d_l)] -> [h, it, i_l, d_l, b16, g, j]
    rel8 = rel8.reshape(H, NT, NBT, IB, N, NG, DG)
    rel8 = np.ascontiguousarray(rel8.transpose(0, 1, 3, 6, 2, 5, 4))
    rel8 = rel8.reshape(H, NT, P, NBT * NG * N)

    in_maps = []
    for h in range(N_CORES):
        sl = slice(h * D, (h + 1) * D)
        wqkv = np.concatenate([Wq[:, sl], Wk[:, sl], Wv[:, sl]], axis=1)
        # bias is added BEFORE RoPE; KF rides in the rope constants
        biasrow = np.concatenate([bq[sl], bk[sl], bv[sl]])
        in_maps.append({
            "xt": xT,
            "wqkv": swz(wqkv).astype(ml_dtypes.bfloat16),
            "biasrow": np.ascontiguousarray(biasrow.reshape(1, 3 * D)
                                            .astype(np.float32)),
            "maskrow": np.ascontiguousarray(maskrow),
            "wo": np.ascontiguousarray(Wo[sl, :]).astype(ml_dtypes.bfloat16),
            "cssn": cssn,
            "tconst": tconst,
            "m512": m512,
            "identf": identf,
            "rel": rel8[h],
        })

    from concourse.bass_utils import run_bass_kernel_spmd
    res = run_bass_kernel_spmd(nc, in_maps, list(range(N_CORES)), trace=TRACE)
    globals()["LAST_RESULT"] = res
    out = np.zeros((N, DIM), dtype=np.float32)
    for h in range(N_CORES):
        out += np.asarray(res.results[h]["out"], dtype=np.float32)
    out += bo[None, :]
    return out.reshape(B, N, DIM)


# revision 44
# speedup vs baseline: 1.0385x; 1.0385x over previous
"""Bass/Trainium2 kernel for nn_Attention_66297115181568 (sparse_attention).

Strategy: head-parallel across 8 NeuronCores; core h computes head h
end-to-end and its 64-row slice of the Wo projection. The host sums the
8 partial (512, 512) outputs (the tensor-parallel all-reduce) and adds bo.

Final version (v8 = 75.1us vs the 76.1us v1 baseline re-measured in
this session; stated harness baseline 91.4us). Key points:
  1. rel-term matmuls in fp8 DoubleRow perf mode (measured 216ns per
     [128, 2, 512] matmul at cruise = 2 fp8/partition/cycle, 2x the
     bf16 col-tiled scheme). DoubleRow is ISA-incompatible with PE
     column tiling, so each (block, g-pair) stationary is a full
     128-column [128, 2, 128] window, zero outside its block strip.
  2. IB=8 block geometry (partitions = 8 i-rows x 16 d): only NG=4
     d-groups -> 4 repl matmuls + 4 strided strip-muls build all the
     stationaries. Windows for the 16 blocks of a tile are overlapped
     in a [1928]-elem plane per (gp, t, u): window bl starts at 120*bl,
     strip bl at 128*bl, so each window holds exactly its own strip
     (at column 8*bl+c) and shared zeros elsewhere.
  3. qpad zero-fill as f32-bitcast memsets (4B/cycle instead of 1),
     split gpsimd/DVE, emitted first so it is off every critical path.
  4. DMA: inputs then all rel pieces (2MB, last one split 2x1MB) on the
     sync queue as one FIFO (sustains ~423 GB/s); only outputs (bf16)
     on the scalar queue, so main-loop ACT work never delays a rel
     doorbell. All rel tiles stay resident (16MB of SBUF).
  5. Softmax: exp(dots/64) with no max-subtraction (logits are O(1));
     the x64 logit scale rides in the k-projection activation (KF);
     exp in 4 column chunks, all 4 w-transposes before the 4 attnV
     matmuls so the PE never stalls on a DVE copy; transposes in bf16.
  6. Projections stay bf16 (an fp8-DoubleRow variant measured SLOWER:
     exposed LDWEIGHTS on half-width stationaries, +8us tensor busy).
"""

import sys

sys.path.insert(0, "/opt/trn_rl_repo")

from contextlib import ExitStack

import numpy as np
import ml_dtypes

import concourse.bass as bass
import concourse.tile as tile
from concourse import mybir
from concourse.ap import AP

# problem dims (hardcoded per spec)
B, N, DIM, H, D = 1, 512, 512, 8, 64
INNER = H * D
N_CORES = 8
P = 128                 # SBUF partitions
NT = N // P             # 4 row tiles
KT = DIM // P           # 4 contraction tiles for projections
IB = 8                  # i-block rows per partition-group
NBT = P // IB           # 16 blocks per row tile
DG = P // IB            # 16 d's per d-group
NG = D // DG            # 4 d-groups
NGP = NG // 2           # 2 d-group pairs (DoubleRow)
WS = P - IB             # 120: window stride in the packed plane
PW = WS * (NBT - 1) + P  # 1928: packed plane width
SCALE = D ** -0.5
NEG_BIG = 1.0e36
RELSCALE = 64.0         # host scales rel by this before fp8 cast
KF = SCALE * RELSCALE   # folded into the k/krot projection activation

f32 = mybir.dt.float32
bf16 = mybir.dt.bfloat16
fp8 = mybir.dt.float8e4
AX = mybir.AxisListType
ALU = mybir.AluOpType
AF = mybir.ActivationFunctionType
PM = mybir.MatmulPerfMode


def legalize_multi_waits(nc):
    """This walrus build supports only one sync-wait per instruction; hoist
    extra waits onto same-engine NoOps placed immediately before."""
    nid = 0
    for fn in nc.m.functions:
        for bb in fn.blocks:
            new = []
            changed = False
            for inst in bb.instructions:
                si = inst.sync_info
                waits = si.on_wait if si is not None else []
                if len(waits) > 1:
                    for w in waits[:-1]:
                        nop = mybir.InstNoOp(name=f"I-waitfix-{nid}")
                        nid += 1
                        nop.engine = inst.engine
                        nop.sync_info = mybir.SyncInfo(on_wait=[w], on_update=[])
                        new.append(nop)
                    si.on_wait = [waits[-1]]
                    inst.sync_info = si
                    changed = True
                new.append(inst)
            if changed:
                bb.instructions = new


def build_nc(use_mask=True, use_bias=False):
    nc = bass.Bass()

    xt_ext = nc.declare_dram_parameter("xt", [P, KT * N], bf16, isOutput=False)
    # combined projection weights [model-dim, (q|k|v)] swizzled
    wqkv_ext = nc.declare_dram_parameter("wqkv", [P, KT * 3 * D], bf16,
                                         isOutput=False)
    biasrow_ext = nc.declare_dram_parameter("biasrow", [1, 3 * D], f32,
                                            isOutput=False)
    maskrow_ext = nc.declare_dram_parameter("maskrow", [1, N], f32,
                                            isOutput=False)
    wo_ext = nc.declare_dram_parameter("wo", [D, DIM], bf16, isOutput=False)
    # RoPE constants in [n, d] layout: [p, nt, {cos, sin_rot}, {q, k}, d];
    # the k half carries KF, sin_rot carries the rotate-half signs
    cssn_ext = nc.declare_dram_parameter("cssn", [P, NT * 2 * 2 * D], bf16,
                                         isOutput=False)
    tconst_ext = nc.declare_dram_parameter("tconst", [D, NG * P], fp8,
                                           isOutput=False)
    m512_ext = nc.declare_dram_parameter("m512", [P, N], bf16, isOutput=False)
    identf_ext = nc.declare_dram_parameter("identf", [P, P], bf16, isOutput=False)
    # rel stream: [tile, p=(i_l*16+d_l), (block16, g4, j)] fp8; 1MB pieces
    rel_ext = nc.declare_dram_parameter("rel", [NT, P, NBT * NG * N], fp8,
                                        isOutput=False)
    out_ext = nc.declare_dram_parameter("out", [N, DIM], bf16, isOutput=True)

    with tile.TileContext(nc) as tc, ExitStack() as ctx:
        dma = nc.sync      # HWDGE queue 1: the whole rel stream
        dma2 = nc.scalar   # HWDGE queue 2: inputs first, then outputs
        consts = ctx.enter_context(tc.tile_pool(name="consts", bufs=1))
        pro = ctx.enter_context(tc.tile_pool(name="pro", bufs=1))
        relp = ctx.enter_context(tc.tile_pool(name="relp", bufs=1))
        smp = ctx.enter_context(tc.tile_pool(name="smp", bufs=2))
        smallp = ctx.enter_context(tc.tile_pool(name="smallp", bufs=2))
        outp = ctx.enter_context(tc.tile_pool(name="outp", bufs=2))
        op = ctx.enter_context(tc.tile_pool(name="op", bufs=1))
        psA = ctx.enter_context(
            tc.tile_pool(name="psA", bufs=2, space=bass.MemorySpace.PSUM))
        psB = ctx.enter_context(
            tc.tile_pool(name="psB", bufs=2, space=bass.MemorySpace.PSUM))
        psW = ctx.enter_context(
            tc.tile_pool(name="psW", bufs=2, space=bass.MemorySpace.PSUM))
        psV = ctx.enter_context(
            tc.tile_pool(name="psV", bufs=1, space=bass.MemorySpace.PSUM))
        psO = ctx.enter_context(
            tc.tile_pool(name="psO", bufs=1, space=bass.MemorySpace.PSUM))

        # ---- qpad zero-fill FIRST: f32-bitcast memsets split across
        # gpsimd and DVE (both otherwise idle at t0) ----
        qpad = consts.tile([P, NGP, 2, NT, PW], fp8, name="qpad")
        half0 = qpad[:, 0:1].rearrange("p a t u w -> p (a t u w)").bitcast(f32)
        half1 = qpad[:, 1:2].rearrange("p a t u w -> p (a t u w)").bitcast(f32)
        nc.gpsimd.memset(half0, 0.0)
        nc.vector.memset(half1, 0.0)

        # ---- inputs on the sync queue, ahead of the rel stream (v1-proven
        # arrangement: one queue, inputs as the FIFO prefix; the scalar
        # queue carries only outputs so main-loop ACT work never delays a
        # transfer doorbell) ----
        xt_sb = pro.tile([P, KT, N], bf16)
        dma.dma_start(out=xt_sb[:], in_=xt_ext.rearrange("p (u n) -> p u n", u=KT))
        wqkv_sb = pro.tile([P, KT, 3 * D], bf16)
        dma.dma_start(out=wqkv_sb[:],
                      in_=wqkv_ext.rearrange("p (u m) -> p u m", u=KT))
        biasrow_sb = consts.tile([1, 3 * D], f32)
        if use_bias:
            dma.dma_start(out=biasrow_sb[:], in_=biasrow_ext[:])
        maskrow_sb = consts.tile([1, N], f32)
        if use_mask:
            dma.dma_start(out=maskrow_sb[:], in_=maskrow_ext[:])
        cssn_sb = consts.tile([P, NT, 2, 2, D], bf16)
        dma.dma_start(out=cssn_sb[:],
                      in_=cssn_ext.rearrange("p (u s c d) -> p u s c d",
                                             u=NT, s=2, c=2))
        tconst_sb = consts.tile([D, NG, P], fp8)
        dma.dma_start(out=tconst_sb[:],
                      in_=tconst_ext.rearrange("d (g p) -> d g p", g=NG))
        m512_sb = consts.tile([P, N], bf16)
        dma.dma_start(out=m512_sb[:], in_=m512_ext[:])
        identb = consts.tile([P, P], bf16)
        dma.dma_start(out=identb[:], in_=identf_ext[:])
        wo_sb = consts.tile([D, DIM], bf16)
        dma.dma_start(out=wo_sb[:], in_=wo_ext[:])
        ones_sb = consts.tile([1, N], f32)
        nc.vector.memset(ones_sb, 1.0)

        # ---- rel stream: 2MB pieces (2 per row tile), all on the sync
        # queue; the final piece split into two 1MB halves for a short
        # PE tail ----
        rel_tiles = {}
        hpt = NBT * NG * N // 2    # free elems per half-tile piece (8 blocks)
        for it in range(NT):
            for pc in range(2):
                if (it, pc) == (NT - 1, 1):
                    rpa = relp.tile([P, hpt // 2], fp8, name="rel7a")
                    dma.dma_start(out=rpa[:],
                                  in_=rel_ext[it, :, pc * hpt:pc * hpt + hpt // 2])
                    rpb = relp.tile([P, hpt // 2], fp8, name="rel7b")
                    dma.dma_start(out=rpb[:],
                                  in_=rel_ext[it, :, pc * hpt + hpt // 2:(pc + 1) * hpt])
                    rel_tiles[(it, pc)] = (rpa, rpb)
                else:
                    rp = relp.tile([P, hpt], fp8, name=f"rel{it}_{pc}")
                    dma.dma_start(out=rp[:],
                                  in_=rel_ext[it, :, pc * hpt:(pc + 1) * hpt])
                    rel_tiles[(it, pc)] = rp

        def rel_mv(it, bl, gp):
            """moving AP [128, 2, 512] for (tile, block16, g-pair)"""
            pc, bi = divmod(bl, NBT // 2)   # half-tile piece, block-in-piece
            rp = rel_tiles[(it, pc)]
            if isinstance(rp, tuple):
                # halves split blocks 8-11 / 12-15 of tile 3
                rp = rp[0] if bi < 4 else rp[1]
                bi = bi % 4
            off = (bi * NG + 2 * gp) * N
            full = rp[:]
            return AP(full.tensor, full.offset + off,
                      [list(full.ap[0]), [N, 2], [1, N]])

        # ---- projections in [n, d] layout: stationary = xT chunk
        # [128, 128], moving = Wqkv [128, 192]. One PSUM bank per n-tile
        # holds q|k|v rows; v copies straight into its attnV layout, q|k
        # copy to SBUF for the DVE RoPE. No qrot/krot projections: the
        # rotate-half happens via strided DVE views. ----
        # Fully pipelined per n-tile: proj (PE) -> copies (ACT) -> RoPE
        # (DVE, rotate-half via d-strided views; sin_rot carries the
        # signs, k rows the KF logit scale) -> transposes back to [d, n].
        qk_nd = pro.tile([P, NT, 2, D], bf16)
        v_sb = pro.tile([P, NT, D], bf16)
        qkp_nd = pro.tile([P, NT, 2, D], bf16)
        t2 = pro.tile([P, NT, 2, D], bf16, tag="ropet2")
        qk_T = pro.tile([D, 2, N], bf16)
        for nt in range(NT):
            ps_q = psA.tile([P, N], f32, tag="big")
            for u in range(KT):
                nc.tensor.matmul(ps_q[:, 0:3 * D],
                                 xt_sb[:, u, nt * P:(nt + 1) * P],
                                 wqkv_sb[:, u, :],
                                 start=(u == 0), stop=(u == KT - 1))
            nc.scalar.copy(qk_nd[:, nt, :, :], ps_q[:, 0:2 * D])
            nc.scalar.copy(v_sb[:, nt, :], ps_q[:, 2 * D:3 * D])
            if use_bias:
                nc.vector.tensor_add(
                    qk_nd[:, nt, :, :], qk_nd[:, nt, :, :],
                    biasrow_sb[0:1, 0:2 * D].rearrange("o (c d) -> o c d", c=2)
                    .partition_broadcast(P))
                nc.vector.tensor_add(
                    v_sb[:, nt, :], v_sb[:, nt, :],
                    biasrow_sb[0:1, 2 * D:3 * D].partition_broadcast(P))
            nc.vector.tensor_mul(qkp_nd[:, nt], qk_nd[:, nt],
                                 cssn_sb[:, nt, 0])
            nc.vector.tensor_mul(t2[:, nt, :, 0::2], qk_nd[:, nt, :, 1::2],
                                 cssn_sb[:, nt, 1, :, 0::2])
            nc.vector.tensor_mul(t2[:, nt, :, 1::2], qk_nd[:, nt, :, 0::2],
                                 cssn_sb[:, nt, 1, :, 1::2])
            nc.vector.tensor_add(qkp_nd[:, nt], qkp_nd[:, nt], t2[:, nt])
            for c in range(2):
                pq = psW.tile([P, P], bf16, tag="tp")
                nc.tensor.transpose(pq[0:D, :], qkp_nd[:, nt, c, :], identb[:])
                nc.scalar.copy(qk_T[:, c, nt * P:(nt + 1) * P], pq[0:D, :])
        qpT = qk_T[:, 0, :]
        kpT = qk_T[:, 1, :]

        # ---- Qpad strips: Rep_g[p, n] = q'T[g*16 + p%16, n]; strips go to
        # plane (gp=g//2, t=g%2): [u, 128*bl + c] (c = p//16 in-strip col),
        # read back as windows [u, 120*bl : 120*bl+128]. ----
        qpad_full = qpad[:]
        ppair = list(qpad_full.ap[0])

        def emit_strip(g):
            ps_rep = psB.tile([P, N], f32, tag="rep")
            nc.tensor.matmul(ps_rep[:], tconst_sb[:, g, :], qpT,
                             start=True, stop=True)
            strip_out = AP(qpad_full.tensor,
                           qpad_full.offset + g * (NT * PW),
                           [ppair, [PW, NT], [P, NBT], [1, IB]])
            nc.vector.tensor_mul(
                strip_out,
                ps_rep.rearrange("p (u v c) -> p u v c", u=NT, v=NBT)[:],
                m512_sb.rearrange("p (u v c) -> p u v c", u=NT, v=NBT)[:])

        # strips first (4 repl + 4 strided muls pipeline PE/DVE; the DVE
        # finishes each pair before the PE's first matmuls need it)
        for g in range(NG):
            emit_strip(g)

        # ---- main loop ----
        o_tiles = [op.tile([P, DIM], bf16, name=f"o{it}") for it in range(NT)]
        for it in range(NT):
            dots_ps = psA.tile([P, N], f32, tag="big")
            nc.tensor.matmul(dots_ps[:], qpT[:, it * P:(it + 1) * P], kpT,
                             start=True, stop=False, skip_group_check=True)
            if use_mask:
                nc.tensor.matmul(dots_ps[:], ones_sb[:, 0:P], maskrow_sb[:],
                                 start=False, stop=False, skip_group_check=True)
            for bl in range(NBT):
                for gp in range(NGP):
                    nc.tensor.matmul(
                        dots_ps[:],
                        qpad[:, gp, :, it, WS * bl:WS * bl + P],
                        rel_mv(it, bl, gp),
                        start=False,
                        stop=(bl == NBT - 1 and gp == NGP - 1),
                        perf_mode=PM.DoubleRow,
                        skip_group_check=True)

            # softmax: unnormalized exp(dots/RELSCALE) in bf16, no
            # max-subtraction; 4 column chunks. All 4 transposes run before
            # the attnV matmuls so the PE never stalls on a DVE copy.
            w_sm = smp.tile([P, N], bf16, tag="w_sm")
            rowsum4 = smallp.tile([P, NT], f32, tag="rowsum4")
            wT_sb = outp.tile([P, NT, P], bf16, tag="wT_sb")
            attn_ps = psV.tile([D, P], f32, tag="attn")
            for jt in range(NT):
                nc.scalar.activation(w_sm[:, jt * P:(jt + 1) * P],
                                     dots_ps[:, jt * P:(jt + 1) * P], AF.Exp,
                                     scale=1.0 / RELSCALE,
                                     accum_out=rowsum4[:, jt:jt + 1])
                wp = psW.tile([P, P], bf16, tag="tp")
                nc.tensor.transpose(wp[:], w_sm[:, jt * P:(jt + 1) * P],
                                    identb[:])
                nc.vector.tensor_copy(wT_sb[:, jt, :], wp[:])
            for jt in range(NT):
                nc.tensor.matmul(attn_ps[:], v_sb[:, jt, :], wT_sb[:, jt, :],
                                 start=(jt == 0), stop=(jt == NT - 1))
            rowsum = smallp.tile([P, 1], f32, tag="rowsum")
            nc.vector.tensor_reduce(rowsum[:], rowsum4[:], AX.X, ALU.add)
            rcp = smallp.tile([P, 1], f32, tag="rcp")
            nc.vector.reciprocal(rcp[:], rowsum[:])

            attn_sb = outp.tile([D, P], bf16, tag="attn_sb")
            nc.scalar.copy(attn_sb[:], attn_ps[:])
            out_ps = psO.tile([P, DIM], f32, tag="out")
            nc.tensor.matmul(out_ps[:], attn_sb[:], wo_sb[:], start=True, stop=True)
            nc.scalar.activation(o_tiles[it][:], out_ps[:], AF.Copy, scale=rcp[:])
            dma2.dma_start(out=out_ext[it * P:(it + 1) * P, :], in_=o_tiles[it][:])

    legalize_multi_waits(nc)
    return nc


_NC_CACHE = None
TRACE = False
LAST_RESULT = None


def _get_nc(use_mask, use_bias):
    global _NC_CACHE
    if _NC_CACHE is None or _NC_CACHE[1] != (use_mask, use_bias):
        _NC_CACHE = (build_nc(use_mask, use_bias), (use_mask, use_bias))
    return _NC_CACHE[0]


def kernel(**inputs):
    x = np.asarray(inputs["x"], dtype=np.float32)
    mask = np.asarray(inputs["mask"])
    rope = np.asarray(inputs["rope"], dtype=np.float32)
    rel_pos = np.asarray(inputs["rel_pos"], dtype=np.float32)
    Wq = np.asarray(inputs["Wq"], dtype=np.float32)
    bq = np.asarray(inputs["bq"], dtype=np.float32)
    Wk = np.asarray(inputs["Wk"], dtype=np.float32)
    bk = np.asarray(inputs["bk"], dtype=np.float32)
    Wv = np.asarray(inputs["Wv"], dtype=np.float32)
    bv = np.asarray(inputs["bv"], dtype=np.float32)
    Wo = np.asarray(inputs["Wo"], dtype=np.float32)
    bo = np.asarray(inputs["bo"], dtype=np.float32)

    use_mask = not bool(np.asarray(mask).all())
    use_bias = bool(np.any(bq) or np.any(bk) or np.any(bv))
    nc = _get_nc(use_mask, use_bias)

    def swz(a):  # [K, M] -> [p, (u, M)] with K = (u, p)
        k, m = a.shape
        return np.ascontiguousarray(
            a.reshape(KT, P, m).transpose(1, 0, 2).reshape(P, KT * m))

    xT = swz(x.reshape(N, DIM).T.astype(np.float32)).astype(ml_dtypes.bfloat16)
    maskrow = ((mask.reshape(1, N).astype(np.float32)) - 1.0) * NEG_BIG

    # cssn[p, nt, {cos, sin_rot}, {q, k}, d]: cos/sin(rope[nt*128+p, d]);
    # the k half carries KF; sin_rot flips sign on even d (rotate-half)
    cos_nd = np.cos(rope).reshape(NT, P, D).transpose(1, 0, 2)  # [P, NT, D]
    sin_nd = np.sin(rope).reshape(NT, P, D).transpose(1, 0, 2)
    sgn = np.where(np.arange(D) % 2 == 0, -1.0, 1.0)[None, None, :]
    cosf = cos_nd[:, :, None, :] * np.array([1.0, KF])[None, None, :, None]
    sinr = (sin_nd * sgn)[:, :, None, :] * np.array([1.0, KF])[None, None, :, None]
    cssn = np.stack([cosf, sinr], axis=2)          # [P, NT, 2, 2, D]
    cssn = cssn.reshape(P, NT * 2 * 2 * D).astype(ml_dtypes.bfloat16)

    # T[d, g, p] = (d == g*DG + p%DG); m512[p, n] = (n%IB == p//DG)
    d_i = np.arange(D)[:, None, None]
    g_i = np.arange(NG)[None, :, None]
    p_i = np.arange(P)[None, None, :]
    tconst = (d_i == g_i * DG + p_i % DG).astype(np.float32)
    tconst = tconst.reshape(D, NG * P).astype(ml_dtypes.float8_e4m3)
    p_2 = np.arange(P)[:, None]
    n_2 = np.arange(N)[None, :]
    m512 = ((n_2 % IB) == (p_2 // DG)).astype(np.float32)
    m512 = m512.astype(ml_dtypes.bfloat16)

    identf = np.eye(P, dtype=np.float32).astype(ml_dtypes.bfloat16)

    # rel: [h, it, p=(i_l*DG+d_l), (block16, g, j)] fp8, scaled by RELSCALE
    rel8 = (rel_pos[0] * RELSCALE).astype(ml_dtypes.float8_e4m3)
    # [h, (it, b16, i_l), j, (g, d_l)] -> [h, it, i_l, d_l, b16, g, j]
    rel8 = rel8.reshape(H, NT, NBT, IB, N, NG, DG)
    rel8 = np.ascontiguousarray(rel8.transpose(0, 1, 3, 6, 2, 5, 4))
    rel8 = rel8.reshape(H, NT, P, NBT * NG * N)

    in_maps = []
    for h in range(N_CORES):
        sl = slice(h * D, (h + 1) * D)
        wqkv = np.concatenate([Wq[:, sl], Wk[:, sl], Wv[:, sl]], axis=1)
        # bias is added BEFORE RoPE; KF rides in the rope constants
        biasrow = np.concatenate([bq[sl], bk[sl], bv[sl]])
        in_maps.append({
            "xt": xT,
            "wqkv": swz(wqkv).astype(ml_dtypes.bfloat16),
            "biasrow": np.ascontiguousarray(biasrow.reshape(1, 3 * D)
                                            .astype(np.float32)),
            "maskrow": np.ascontiguousarray(maskrow),
            "wo": np.ascontiguousarray(Wo[sl, :]).astype(ml_dtypes.bfloat16),
            "cssn": cssn,
            "tconst": tconst,
            "m512": m512,
            "identf": identf,
            "rel": rel8[h],
        })

    from concourse.bass_utils import run_bass_kernel_spmd
    res = run_bass_kernel_spmd(nc, in_maps, list(range(N_CORES)), trace=TRACE)
    globals()["LAST_RESULT"] = res
    out = np.zeros((N, DIM), dtype=np.float32)
    for h in range(N_CORES):
        out += np.asarray(res.results[h]["out"], dtype=np.float32)
    out += bo[None, :]
    return out.reshape(B, N, DIM)
